# revision 1
# baseline (speedup 1.0000x reference)
"""DWT (db4-style, depthwise stride-2, reflect-pad) layer as a Trainium2
Bass/Tile kernel.

Math: for input x[B, T, C] and 8-tap filters lo/hi the reference computes a
reflect-pad-7, stride-2, depthwise cross-correlation cropped by 3 per side:

    out[b, t', c]     = sum_k lo[k] * xe[b, 2 t' + k, c]
    out[b, t', C + c] = sum_k hi[k] * xe[b, 2 t' + k, c]

with xe[u] = x[u - 1] for u in [1, T+1), xe[0] = x[1], xe[T+1] = x[T-2]
(after the crop only one reflected element is needed per side), and
t' in [0, T/2 - 2).

Device mapping (per core):
  - host pre-transposes x to [T, Bl, C] so DMA partition lines are 2 KB
    contiguous; the output is produced as [T', Bl, 2C] (2 KB per t' row)
    and transposed back on the host.
  - time on the SBUF partition axis, 2 steps per partition (polyphase):
    X[p, (j, b, c)] = xe[u0 + 2p + j].  For each filter f the banded
    stationary matrices W_f_j[p, m] = f[2 (p - m) + j] (p - m in [0, 4))
    accumulate over j in PSUM: 125 outputs t' = u0/2 + m per tile.
  - PSUM lo/hi [125, Bl*C] are interleaved into [t', (b, lo|hi, c)] in
    SBUF by two DVE copies, then stored with ONE DMA per tile
    ([125 partitions x 2 KB]).
  - loads go via HWDGE (spread across all 16 SDMA engines); stores
    alternate SWDGE/HWDGE per tile: HWDGE stores are pinned to a 5-engine
    SDMA subset, SWDGE stores spread but cost GpSimd Q7 descriptor-
    generation time, so splitting balances the two store paths.

Sharding: data-parallel over batch, 4 batches per core on 8 cores.
"""

import math

import numpy as np

import concourse.bacc as bacc
import concourse.mybir as mybir
import concourse.tile as tile
from concourse.bass_utils import run_bass_kernel_spmd

F32 = mybir.dt.float32

B, T, C = 32, 16384, 64
N_CORES = 8
BL = B // N_CORES  # 4 batches per core
M = 125            # output positions per 128-partition tile


def _build_nc(Bl: int, Tn: int, Cn: int, sw_frac: float = 1.0):
    """Single-core program for x_dev[Tn, Bl, Cn] -> out_dev[Tn/2-2, Bl, 2*Cn]."""
    nout = Tn // 2 - 2
    ntiles = math.ceil(nout / M)
    nfree = Bl * Cn

    nc = bacc.Bacc("TRN2", target_bir_lowering=False, debug=False)
    x_d = nc.dram_tensor("x", [Tn, Bl, Cn], F32, kind="ExternalInput")
    w_d = nc.dram_tensor("w", [4, 128, M], F32, kind="ExternalInput")
    o_d = nc.dram_tensor("out", [nout, Bl, 2 * Cn], F32, kind="ExternalOutput")

    with tile.TileContext(nc) as tc:
        with (
            tc.tile_pool(name="wpool", bufs=1) as wpool,
            tc.tile_pool(name="xin", bufs=6) as xpool,
            tc.tile_pool(name="oout", bufs=4) as opool,
            tc.tile_pool(name="ps", bufs=4, space="PSUM") as pspool,
        ):
            # Stationary banded matrices: [lo_j0, lo_j1, hi_j0, hi_j1]
            w_t = wpool.tile([128, 4 * M], F32)
            nc.sync.dma_start(out=w_t[:].rearrange("p (f m) -> p f m", f=4),
                              in_=w_d.rearrange("f p m -> p f m"))
            w_ap = [w_t[:, f * M:(f + 1) * M] for f in range(4)]

            def emit_pair(i):
                # two full middle tiles (i, i+1) in one matmul set: moving
                # free dim (h, b, c) = 512 (fp32 max), halving MM count
                t0 = M * i
                u0 = 2 * t0
                xt = xpool.tile([128, 4 * nfree], F32, tag="xt")
                for h in range(2):
                    uh = u0 + 250 * h
                    nc.sync.dma_start(
                        out=xt[:, 2 * nfree * h:2 * nfree * (h + 1)],
                        in_=x_d[uh - 1:uh + 255].rearrange("(p j) b c -> p (j b c)", j=2))
                xv = xt[:].rearrange("p (h j w) -> p h j w", h=2, j=2)
                x0 = xv[:, :, 0, :]
                x1 = xv[:, :, 1, :]
                ps_lo = pspool.tile([M, 2 * nfree], F32, tag="ps")
                ps_hi = pspool.tile([M, 2 * nfree], F32, tag="ps")
                nc.tensor.matmul(out=ps_lo[:], lhsT=w_ap[0], rhs=x0, start=True, stop=False)
                nc.tensor.matmul(out=ps_lo[:], lhsT=w_ap[1], rhs=x1, start=False, stop=True)
                nc.tensor.matmul(out=ps_hi[:], lhsT=w_ap[2], rhs=x0, start=True, stop=False)
                nc.tensor.matmul(out=ps_hi[:], lhsT=w_ap[3], rhs=x1, start=False, stop=True)
                ot = opool.tile([128, 2 * Bl * 2 * Cn], F32, tag="ot")
                ov = ot[:].rearrange("p (h b f c) -> p h b f c", h=2, b=Bl, f=2)
                nc.vector.tensor_copy(
                    out=ov[0:M, :, :, 0, :],
                    in_=ps_lo[:].rearrange("p (h b c) -> p h b c", h=2, b=Bl))
                nc.vector.tensor_copy(
                    out=ov[0:M, :, :, 1, :],
                    in_=ps_hi[:].rearrange("p (h b c) -> p h b c", h=2, b=Bl))
                nc.gpsimd.dma_start(
                    out=o_d[t0:t0 + 2 * M].rearrange("(h t) b c -> t h (b c)", h=2),
                    in_=ot[0:M].rearrange("p (h w) -> p h w", h=2))

            if ntiles >= 6:
                n_pair_tiles = ((ntiles - 4) // 2) * 2
                singles = [0, 1] + list(range(2 + n_pair_tiles, ntiles))
                pairs = list(range(2, 2 + n_pair_tiles, 2))
            else:
                singles, pairs = list(range(ntiles)), []
            sched = sorted([(i, False) for i in singles] + [(i, True) for i in pairs])

            for i, is_pair in sched:
                if is_pair:
                    emit_pair(i)
                    continue
                t0 = M * i
                nvalid = min(M, nout - t0)
                u0 = 2 * t0

                xt = xpool.tile([128, 2 * nfree], F32, tag="xt")
                ld = nc.sync
                if i == 0:
                    # partition 0: j=0 <- xe[0] = x[1], j=1 <- xe[1] = x[0]
                    ld.dma_start(out=xt[0:1, 0:nfree],
                                      in_=x_d[1:2].rearrange("t b c -> t (b c)"))
                    ld.dma_start(out=xt[0:1, nfree:2 * nfree],
                                      in_=x_d[0:1].rearrange("t b c -> t (b c)"))
                    nc.sync.dma_start(
                        out=xt[1:128],
                        in_=x_d[1:255].rearrange("(p j) b c -> p (j b c)", j=2))
                elif i == ntiles - 1:
                    nc.vector.memset(xt[:], 0.0)
                    pfull = (Tn - u0) // 2
                    nc.sync.dma_start(
                        out=xt[0:pfull],
                        in_=x_d[u0 - 1:u0 - 1 + 2 * pfull]
                        .rearrange("(p j) b c -> p (j b c)", j=2))
                    # tail partition: j=0 <- x[Tn-1], j=1 <- xe[Tn+1] = x[Tn-2]
                    ld.dma_start(out=xt[pfull:pfull + 1, 0:nfree],
                                      in_=x_d[Tn - 1:Tn].rearrange("t b c -> t (b c)"))
                    ld.dma_start(out=xt[pfull:pfull + 1, nfree:2 * nfree],
                                      in_=x_d[Tn - 2:Tn - 1].rearrange("t b c -> t (b c)"))
                else:
                    ld.dma_start(
                        out=xt[:],
                        in_=x_d[u0 - 1:u0 + 255].rearrange("(p j) b c -> p (j b c)", j=2))

                x0 = xt[:, 0:nfree]
                x1 = xt[:, nfree:2 * nfree]
                ps_lo = pspool.tile([M, nfree], F32, tag="ps")
                ps_hi = pspool.tile([M, nfree], F32, tag="ps")
                nc.tensor.matmul(out=ps_lo[:], lhsT=w_ap[0], rhs=x0, start=True, stop=False)
                nc.tensor.matmul(out=ps_lo[:], lhsT=w_ap[1], rhs=x1, start=False, stop=True)
                nc.tensor.matmul(out=ps_hi[:], lhsT=w_ap[2], rhs=x0, start=True, stop=False)
                nc.tensor.matmul(out=ps_hi[:], lhsT=w_ap[3], rhs=x1, start=False, stop=True)

                ot = opool.tile([128, Bl * 2 * Cn], F32)
                ov = ot[:].rearrange("p (b f c) -> p b f c", b=Bl, f=2)
                nc.vector.tensor_copy(
                    out=ov[0:nvalid, :, 0, :],
                    in_=ps_lo[0:nvalid].rearrange("p (b c) -> p b c", b=Bl))
                nc.vector.tensor_copy(
                    out=ov[0:nvalid, :, 1, :],
                    in_=ps_hi[0:nvalid].rearrange("p (b c) -> p b c", b=Bl))

                store_eng = nc.gpsimd if (i % 4) < round(sw_frac * 4) else nc.sync
                store_eng.dma_start(
                    out=o_d[t0:t0 + nvalid].rearrange("t b c -> t (b c)"),
                    in_=ot[0:nvalid])

    nc.compile()
    return nc


def _build_w(dec_lo: np.ndarray, dec_hi: np.ndarray) -> np.ndarray:
    """Banded stationary matrices [4, 128, M]: order lo_j0, lo_j1, hi_j0, hi_j1."""
    lo = np.asarray(dec_lo, np.float32)
    hi = np.asarray(dec_hi, np.float32)
    w = np.zeros((4, 128, M), np.float32)
    for m in range(M):
        for d in range(4):
            w[0, m + d, m] = lo[2 * d]
            w[1, m + d, m] = lo[2 * d + 1]
            w[2, m + d, m] = hi[2 * d]
            w[3, m + d, m] = hi[2 * d + 1]
    return w


_NC_CACHE = {}


def _get_nc():
    key = (BL, T, C)
    if key not in _NC_CACHE:
        _NC_CACHE[key] = _build_nc(*key)
    return _NC_CACHE[key]


def kernel(x: np.ndarray, dec_lo: np.ndarray, dec_hi: np.ndarray) -> np.ndarray:
    x = np.asarray(x, np.float32)
    assert x.shape == (B, T, C), x.shape
    nc = _get_nc()
    w = _build_w(dec_lo, dec_hi)
    in_maps = [
        {"x": np.ascontiguousarray(x[i * BL:(i + 1) * BL].transpose(1, 0, 2)),
         "w": w}
        for i in range(N_CORES)
    ]
    res = run_bass_kernel_spmd(nc, in_maps, core_ids=list(range(N_CORES)))
    # device output is [T', Bl, 2C] per core -> back to [B, T', 2C]
    return np.concatenate(
        [res.results[i]["out"].transpose(1, 0, 2) for i in range(N_CORES)], axis=0)



# revision 2
# speedup vs baseline: 1.2276x; 1.2276x over previous
"""DWT (db4-style, depthwise stride-2, reflect-pad) layer as a Trainium2
Bass/Tile kernel — bf16 datapath version.

Math: for input x[B, T, C] and 8-tap filters lo/hi the reference computes a
reflect-pad-7, stride-2, depthwise cross-correlation cropped by 3 per side:

    out[b, t', c]     = sum_k lo[k] * xe[b, 2 t' + k, c]
    out[b, t', C + c] = sum_k hi[k] * xe[b, 2 t' + k, c]

with xe[u] = x[u - 1] for u in [1, T+1), xe[0] = x[1], xe[T+1] = x[T-2]
(after the crop only one reflected element is needed per side), and
t' in [0, T/2 - 2).

Strategy (all per core; data-parallel over batch, Bl = 4 batches/core):
  - The HOST builds the exact overlapped device tile layout in bf16:
    x_dev[i, p, (h, w)] = xe[2*t0_i + 122*h + p][b, c], h in [0,4), so every
    device load is a plain contiguous [128 x 2 KB] DMA.  bf16 halves HBM
    traffic vs fp32 and runs the PE at 1 cycle/row instead of 4.
  - One stationary banded matrix W[p, s*61+q] = f_s[p - 2q] (s = lo|hi,
    p-2q in [0,8)) computes 61 outputs per 128-element window for both
    filters in a single matmul; the window index h lives purely in the rhs
    free dimension, so each super-tile is 2 matmuls of moving-free 512
    into a 2-bank PSUM tile [122, 1024] fp32.
  - One engine copy (alternating DVE / Activation) downcasts PSUM fp32 ->
    SBUF bf16 [122, 2 KB]; one DMA stores it (alternating Activation-HWDGE
    / GpSimd-SWDGE so no single sequencer serializes: each dma_start costs
    ~0.6 us on its issuing engine).
  - The last super-tile overlaps the previous one (t0 = T' - 244) so all 34
    super-tiles are identical; overlapping rows are recomputed bitwise
    identically, making the racing stores benign.
  - The host un-permutes [i, (s q), (h w)] -> [b, t', 2C] and upcasts.
"""

import numpy as np
import ml_dtypes

import concourse.bacc as bacc
import concourse.mybir as mybir
import concourse.tile as tile
from concourse.bass_utils import run_bass_kernel_spmd

F32 = mybir.dt.float32
BF16 = mybir.dt.bfloat16
BF16_NP = ml_dtypes.bfloat16

B, T, C = 32, 16384, 64
N_CORES = 8
BL = B // N_CORES           # 4 batches per core
NF = BL * C                 # 256 moving elements per time row
Q = 61                      # outputs per 128-wide window
H = 4                       # windows per super-tile
SUP = Q * H                 # 244 t' per super-tile
TP = T // 2 - 2             # 8190 output positions
NSUP = (TP + SUP - 1) // SUP  # 34 super-tiles (last one overlaps)
T0S = [SUP * i for i in range(NSUP - 1)] + [TP - SUP]


def _build_nc():
    """Single-core program: x_dev[NSUP,128,H*NF] bf16 -> o_dev[NSUP,122,H*NF] bf16."""
    nc = bacc.Bacc("TRN2", target_bir_lowering=False, debug=False)
    x_d = nc.dram_tensor("x", [NSUP, 128, H * NF], BF16, kind="ExternalInput")
    w_d = nc.dram_tensor("w", [128, 2 * Q], BF16, kind="ExternalInput")
    o_d = nc.dram_tensor("out", [NSUP, 2 * Q, H * NF], BF16, kind="ExternalOutput")

    with tile.TileContext(nc) as tc:
        with (
            tc.tile_pool(name="wpool", bufs=1) as wpool,
            tc.tile_pool(name="xin", bufs=6) as xpool,
            tc.tile_pool(name="oout", bufs=6) as opool,
            tc.tile_pool(name="ps", bufs=4, space="PSUM") as pspool,
        ):
            w_t = wpool.tile([128, 2 * Q], BF16)
            nc.sync.dma_start(out=w_t[:], in_=w_d[:])

            for i in range(NSUP):
                xt = xpool.tile([128, H * NF], BF16, tag="xt")
                nc.sync.dma_start(out=xt[:], in_=x_d[i])
                ps = pspool.tile([2 * Q, H * NF], F32, tag="ps")
                half = H * NF // 2  # 512 moving rows per matmul (one PSUM bank)
                for m in range(2):
                    nc.tensor.matmul(out=ps[:, half * m:half * (m + 1)],
                                     lhsT=w_t[:], rhs=xt[:, half * m:half * (m + 1)],
                                     start=True, stop=True)
                ot = opool.tile([2 * Q, H * NF], BF16, tag="ot")
                if i % 2 == 0:
                    nc.vector.tensor_copy(out=ot[:], in_=ps[:])
                    nc.scalar.dma_start(out=o_d[i], in_=ot[:])
                else:
                    nc.scalar.copy(out=ot[:], in_=ps[:])
                    nc.gpsimd.dma_start(out=o_d[i], in_=ot[:])

    nc.compile()
    return nc


def _build_w(dec_lo: np.ndarray, dec_hi: np.ndarray) -> np.ndarray:
    """Banded stationary matrix [128, 2Q] bf16: cols [lo q=0..60 | hi q=0..60]."""
    w = np.zeros((128, 2 * Q), np.float32)
    for s, f in enumerate((np.asarray(dec_lo, np.float32),
                           np.asarray(dec_hi, np.float32))):
        for q in range(Q):
            w[2 * q:2 * q + 8, s * Q + q] = f
    return w.astype(BF16_NP)


def _prep_x(x: np.ndarray) -> list[np.ndarray]:
    """Per-core overlapped window layout [NSUP, 128, H*NF] bf16."""
    xb = x.astype(BF16_NP)
    t0 = np.asarray(T0S)
    # window start xe-index per (super, h, p)
    idx = (2 * t0[:, None, None] + 122 * np.arange(H)[None, :, None]
           + np.arange(128)[None, None, :])                    # [NSUP, H, 128]
    out = []
    for core in range(N_CORES):
        xc = np.ascontiguousarray(
            xb[core * BL:(core + 1) * BL].transpose(1, 0, 2))  # [T, BL, C]
        xe = np.concatenate([xc[1:2], xc, xc[T - 2:T - 1]], axis=0)  # [T+2, BL, C]
        xw = xe.reshape(T + 2, NF)[idx]                        # [NSUP, H, 128, NF]
        out.append(np.ascontiguousarray(
            xw.transpose(0, 2, 1, 3)).reshape(NSUP, 128, H * NF))
    return out


_NC_CACHE = {}


def _get_nc():
    if "nc" not in _NC_CACHE:
        _NC_CACHE["nc"] = _build_nc()
    return _NC_CACHE["nc"]


def kernel(x: np.ndarray, dec_lo: np.ndarray, dec_hi: np.ndarray) -> np.ndarray:
    x = np.asarray(x, np.float32)
    assert x.shape == (B, T, C), x.shape
    nc = _get_nc()
    w = _build_w(dec_lo, dec_hi)
    in_maps = [{"x": xc, "w": w} for xc in _prep_x(x)]
    res = run_bass_kernel_spmd(nc, in_maps, core_ids=list(range(N_CORES)))
    out = np.empty((B, TP, 2 * C), np.float32)
    for core in range(N_CORES):
        r = np.asarray(res.results[core]["out"]).reshape(NSUP, 2, Q, H, BL, C)
        # (i, s, q, h, b, c) -> [b, 244i + 61h + q, s, c]
        main = r[:NSUP - 1].transpose(4, 0, 3, 2, 1, 5).reshape(
            BL, (NSUP - 1) * SUP, 2 * C)
        last = r[NSUP - 1].transpose(3, 2, 1, 0, 4).reshape(BL, SUP, 2 * C)
        oc = out[core * BL:(core + 1) * BL]
        oc[:, :(NSUP - 1) * SUP] = main
        oc[:, TP - SUP:] = last
    return out
